# revision 4
# baseline (speedup 1.0000x reference)
"""nn_Linear8bit on 8 TRN2 NeuronCores — column-parallel (tensor-parallel on out_features).

out[m, n] = sum_k x[m, k] * wq[n, k] * scale[n] + bias[n]
  x: [2, 512, 4096] f32, wq: [16384, 4096] int32 (int8-valued), scale/bias: [16384] f32

Sharding: W/scale/bias row-sharded 2048/core; x replicated; no collectives.
Per-core dataflow:
  - x: gpsimd cast-DMA f32->bf16 (cast in SDMA datapath), xbar DMA-transpose to
    xT[kp, mt, kt, m] (k on partitions), resident in SBUF for the whole kernel.
  - per n-tile (128 rows of W): cast-DMA int32->bf16, xbar-transpose to wT[kp, kt, n],
    32 accumulating matmuls per 512-token chunk (lhsT=wT k-tile, rhs=xT slice),
    PSUM f32 evicted via one DVE tensor_scalar (x*scale + bias, both per-partition),
    DMA out as out.T [2048, 1024] f32.
  - host: concat core outputs along n, transpose to [1024, 16384].
"""

import numpy as np

import concourse.tile as tile
from concourse import bacc, mybir
from concourse.bass_utils import run_bass_kernel_spmd

B, S, K, N = 2, 512, 4096, 16384
M = B * S              # 1024 tokens
NCORES = 8
NSH = N // NCORES      # 2048 out-features per core
P = 128
KT = K // P            # 32 k-tiles
MT = M // P            # 8 m-tiles
NT = NSH // P          # 16 n-tiles per core
MCW = 512              # moving free dim per matmul
MCH = M // MCW         # 2 token chunks


def build(w_bufs: int = 3, psum_bufs: int = 2):
    nc = bacc.Bacc("TRN2", target_bir_lowering=False, debug=False)
    x_d = nc.dram_tensor("x", [M, K], mybir.dt.float32, kind="ExternalInput")
    w_d = nc.dram_tensor("wq", [NSH, K], mybir.dt.int32, kind="ExternalInput")
    s_d = nc.dram_tensor("scale", [NSH, 1], mybir.dt.float32, kind="ExternalInput")
    b_d = nc.dram_tensor("bias", [NSH, 1], mybir.dt.float32, kind="ExternalInput")
    o_d = nc.dram_tensor("outT", [NSH, M], mybir.dt.float32, kind="ExternalOutput")

    with tile.TileContext(nc) as tc:
        with (
            tc.tile_pool(name="xstage", bufs=2) as xstage_pool,
            tc.tile_pool(name="xT", bufs=1) as xT_pool,
            tc.tile_pool(name="wstage", bufs=w_bufs) as wstage_pool,
            tc.tile_pool(name="wT", bufs=w_bufs) as wT_pool,
            tc.tile_pool(name="small", bufs=3) as small_pool,
            tc.tile_pool(name="osb", bufs=4) as osb_pool,
            tc.tile_pool(name="psum", bufs=psum_bufs, space="PSUM") as psum_pool,
        ):
            # x prep: cast + transpose, stays resident. Layout [kp, mt, kt, m_in]
            # so each xbar dest slice [:, mt] is per-partition contiguous.
            xT = xT_pool.tile([P, MT, KT, P], mybir.dt.bfloat16)
            for mt in range(MT):
                x_sb = xstage_pool.tile([P, K], mybir.dt.bfloat16, tag="x_sb")
                nc.gpsimd.dma_start(out=x_sb[:], in_=x_d.ap()[mt * P:(mt + 1) * P, :])
                nc.sync.dma_start(out=xT[:, mt], in_=x_sb[:], transpose=True)

            for nt in range(NT):
                w_sb = wstage_pool.tile([P, K], mybir.dt.bfloat16, tag="w_sb")
                nc.gpsimd.dma_start(out=w_sb[:], in_=w_d.ap()[nt * P:(nt + 1) * P, :])
                wT = wT_pool.tile([P, KT, P], mybir.dt.bfloat16, tag="wT")
                nc.sync.dma_start(out=wT[:], in_=w_sb[:], transpose=True)

                s_sb = small_pool.tile([P, 1], mybir.dt.float32, tag="s_sb")
                nc.sync.dma_start(out=s_sb[:], in_=s_d.ap()[nt * P:(nt + 1) * P, :])
                b_sb = small_pool.tile([P, 1], mybir.dt.float32, tag="b_sb")
                nc.sync.dma_start(out=b_sb[:], in_=b_d.ap()[nt * P:(nt + 1) * P, :])

                pss = [
                    psum_pool.tile(
                        [P, MCW], mybir.dt.float32, name=f"ps{c}", tag=f"ps{c}"
                    )
                    for c in range(MCH)
                ]
                # k-outer / chunk-inner: each stationary k-tile serves both chunks.
                for kt in range(KT):
                    for c in range(MCH):
                        nc.tensor.matmul(
                            pss[c][:],
                            wT[:, kt, :],
                            xT[:, 4 * c:4 * (c + 1), kt, :],
                            start=(kt == 0),
                            stop=(kt == KT - 1),
                        )
                for c in range(MCH):
                    o_sb = osb_pool.tile([P, MCW], mybir.dt.float32, tag="o_sb")
                    nc.vector.tensor_scalar(
                        out=o_sb[:],
                        in0=pss[c][:],
                        scalar1=s_sb[:],
                        scalar2=b_sb[:],
                        op0=mybir.AluOpType.mult,
                        op1=mybir.AluOpType.add,
                    )
                    nc.sync.dma_start(
                        out=o_d.ap()[nt * P:(nt + 1) * P, c * MCW:(c + 1) * MCW],
                        in_=o_sb[:],
                    )
    nc.compile()
    return nc


def make_in_maps(x, weight_quant, scale, bias):
    x2 = np.ascontiguousarray(x.reshape(M, K)).astype(np.float32, copy=False)
    scale = np.asarray(scale, dtype=np.float32).reshape(N, 1)
    bias = np.asarray(bias, dtype=np.float32).reshape(N, 1)
    wq = np.asarray(weight_quant, dtype=np.int32)
    in_maps = []
    for i in range(NCORES):
        sl = slice(i * NSH, (i + 1) * NSH)
        in_maps.append({
            "x": x2,
            "wq": np.ascontiguousarray(wq[sl]),
            "scale": np.ascontiguousarray(scale[sl]),
            "bias": np.ascontiguousarray(bias[sl]),
        })
    return in_maps


def gather_output(results):
    outT = np.concatenate([np.asarray(r["outT"]) for r in results], axis=0)  # [N, M]
    return np.ascontiguousarray(outT.T).reshape(B, S, N).astype(np.float32, copy=False)


def kernel(x, weight_quant, scale, bias):
    nc = build()
    in_maps = make_in_maps(x, weight_quant, scale, bias)
    res = run_bass_kernel_spmd(nc, in_maps, core_ids=list(range(NCORES)))
    return gather_output(res.results)


if __name__ == "__main__":
    rng = np.random.default_rng(0)
    x = rng.standard_normal((B, S, K), dtype=np.float32)
    wq = rng.integers(-128, 128, size=(N, K), dtype=np.int64).astype(np.int32)
    scale = rng.uniform(0.001, 0.02, size=(N,)).astype(np.float32)
    bias = rng.standard_normal((N,), dtype=np.float32)
    out = kernel(x=x, weight_quant=wq, scale=scale, bias=bias)
    w = wq.astype(np.float32) * scale[:, None]
    exp = x.reshape(M, K) @ w.T + bias
    err = np.abs(out.reshape(M, N) - exp).max() / np.abs(exp).max()
    print("self-check rel err:", err)


# revision 5
# speedup vs baseline: 1.1608x; 1.1608x over previous
"""nn_Linear8bit on 8 TRN2 NeuronCores — column-parallel (tensor-parallel on out_features).

out[m, n] = sum_k x[m, k] * wq[n, k] * scale[n] + bias[n]
  x: [2, 512, 4096] f32, wq: [16384, 4096] int32 (int8-valued), scale/bias: [16384] f32

Sharding: W/scale/bias row-sharded 2048/core; x replicated (fed k-major as part of
layout prep); no collectives.

Per-core dataflow:
  - x.T (k-major f32) -> gpsimd cast-DMA f32->bf16 straight into resident SBUF
    tiles xT[kp, kt, m]  (contraction dim on partitions).
  - per n-tile (128 rows of W): gpsimd cast-DMA int32->bf16 (SDMA casts in the
    datapath), xbar DMA-transpose (Sync engine, transposes only -> no xbar/copy
    mode transitions) to wT[kp, kt, n].
  - 2 x 32 accumulating matmuls per n-tile (k-inner, one PSUM bank per 512-token
    chunk), PSUM f32 evicted via one DVE tensor_scalar (x*scale + bias, both
    per-partition scalars), output written as out.T [2048, 1024] f32 on Scalar
    HWDGE (keeps Sync xbar-only).
  - host: concat core outputs along n, transpose to [1024, 16384].
"""

import numpy as np

import concourse.tile as tile
from concourse import bacc, mybir
from concourse.bass_utils import run_bass_kernel_spmd

B, S, K, N = 2, 512, 4096, 16384
M = B * S              # 1024 tokens
NCORES = 8
NSH = N // NCORES      # 2048 out-features per core
P = 128
KT = K // P            # 32 k-tiles
NT = NSH // P          # 16 n-tiles per core
MCW = 512              # moving free dim per matmul (= one PSUM bank of f32)
MCH = M // MCW         # 2 token chunks
XG = 8                 # x load groups (4 k-tiles per DMA)


def build(w_bufs: int = 3, psum_bufs: int = 3):
    nc = bacc.Bacc("TRN2", target_bir_lowering=False, debug=False)
    xT_d = nc.dram_tensor("xT", [K, M], mybir.dt.float32, kind="ExternalInput")
    w_d = nc.dram_tensor("wq", [NSH, K], mybir.dt.int32, kind="ExternalInput")
    s_d = nc.dram_tensor("scale", [NSH, 1], mybir.dt.float32, kind="ExternalInput")
    b_d = nc.dram_tensor("bias", [NSH, 1], mybir.dt.float32, kind="ExternalInput")
    o_d = nc.dram_tensor("outT", [NSH, M], mybir.dt.float32, kind="ExternalOutput")

    kt_per_g = KT // XG
    with tile.TileContext(nc) as tc:
        with (
            tc.tile_pool(name="xT_pool", bufs=1) as xT_pool,
            tc.tile_pool(name="wstage", bufs=w_bufs) as wstage_pool,
            tc.tile_pool(name="wT_pool", bufs=w_bufs) as wT_pool,
            tc.tile_pool(name="small", bufs=4) as small_pool,
            tc.tile_pool(name="osb", bufs=4) as osb_pool,
            tc.tile_pool(name="psum", bufs=psum_bufs, space="PSUM") as psum_pool,
        ):
            # x: cast-DMA straight into transposed-resident layout (k on partitions)
            xT = xT_pool.tile([P, KT, M], mybir.dt.bfloat16)
            for g in range(XG):
                nc.gpsimd.dma_start(
                    out=xT[:, g * kt_per_g:(g + 1) * kt_per_g, :],
                    in_=xT_d.ap()[g * kt_per_g * P:(g + 1) * kt_per_g * P, :].rearrange(
                        "(kt p) m -> p kt m", p=P
                    ),
                )

            for nt in range(NT):
                w_sb = wstage_pool.tile([P, K], mybir.dt.bfloat16, tag="w_sb")
                nc.gpsimd.dma_start(out=w_sb[:], in_=w_d.ap()[nt * P:(nt + 1) * P, :])
                wT = wT_pool.tile([P, KT, P], mybir.dt.bfloat16, tag="wT")
                nc.sync.dma_start(out=wT[:], in_=w_sb[:], transpose=True)

                s_sb = small_pool.tile([P, 1], mybir.dt.float32, tag="s_sb")
                nc.scalar.dma_start(out=s_sb[:], in_=s_d.ap()[nt * P:(nt + 1) * P, :])
                b_sb = small_pool.tile([P, 1], mybir.dt.float32, tag="b_sb")
                nc.scalar.dma_start(out=b_sb[:], in_=b_d.ap()[nt * P:(nt + 1) * P, :])

                for c in range(MCH):
                    ps = psum_pool.tile(
                        [P, MCW], mybir.dt.float32, name=f"ps{c}", tag=f"ps{c}"
                    )
                    # k-inner: 32 back-to-back accumulating matmuls on one bank,
                    # 2D contiguous moving operand.
                    for kt in range(KT):
                        nc.tensor.matmul(
                            ps[:],
                            wT[:, kt, :],
                            xT[:, kt, c * MCW:(c + 1) * MCW],
                            start=(kt == 0),
                            stop=(kt == KT - 1),
                        )
                    o_sb = osb_pool.tile([P, MCW], mybir.dt.float32, tag="o_sb")
                    nc.vector.tensor_scalar(
                        out=o_sb[:],
                        in0=ps[:],
                        scalar1=s_sb[:],
                        scalar2=b_sb[:],
                        op0=mybir.AluOpType.mult,
                        op1=mybir.AluOpType.add,
                    )
                    nc.scalar.dma_start(
                        out=o_d.ap()[nt * P:(nt + 1) * P, c * MCW:(c + 1) * MCW],
                        in_=o_sb[:],
                    )
    nc.compile()
    return nc


def make_in_maps(x, weight_quant, scale, bias):
    x2T = np.ascontiguousarray(
        np.asarray(x, dtype=np.float32).reshape(M, K).T
    )  # [K, M] k-major replica
    scale = np.asarray(scale, dtype=np.float32).reshape(N, 1)
    bias = np.asarray(bias, dtype=np.float32).reshape(N, 1)
    wq = np.asarray(weight_quant, dtype=np.int32)
    in_maps = []
    for i in range(NCORES):
        sl = slice(i * NSH, (i + 1) * NSH)
        in_maps.append({
            "xT": x2T,
            "wq": np.ascontiguousarray(wq[sl]),
            "scale": np.ascontiguousarray(scale[sl]),
            "bias": np.ascontiguousarray(bias[sl]),
        })
    return in_maps


def gather_output(results):
    outT = np.concatenate([np.asarray(r["outT"]) for r in results], axis=0)  # [N, M]
    return np.ascontiguousarray(outT.T).reshape(B, S, N).astype(np.float32, copy=False)


def kernel(x, weight_quant, scale, bias):
    nc = build()
    in_maps = make_in_maps(x, weight_quant, scale, bias)
    res = run_bass_kernel_spmd(nc, in_maps, core_ids=list(range(NCORES)))
    return gather_output(res.results)


if __name__ == "__main__":
    rng = np.random.default_rng(0)
    x = rng.standard_normal((B, S, K), dtype=np.float32)
    wq = rng.integers(-128, 128, size=(N, K), dtype=np.int64).astype(np.int32)
    scale = rng.uniform(0.001, 0.02, size=(N,)).astype(np.float32)
    bias = rng.standard_normal((N,), dtype=np.float32)
    out = kernel(x=x, weight_quant=wq, scale=scale, bias=bias)
    w = wq.astype(np.float32) * scale[:, None]
    exp = x.reshape(M, K) @ w.T + bias
    err = np.abs(out.reshape(M, N) - exp).max() / np.abs(exp).max()
    print("self-check rel err:", err)


# revision 7
# speedup vs baseline: 1.1835x; 1.0196x over previous
"""nn_Linear8bit on 8 TRN2 NeuronCores — column-parallel (tensor-parallel on out_features).

out[m, n] = sum_k x[m, k] * wq[n, k] * scale[n] + bias[n]
  x: [2, 512, 4096] f32, wq: [16384, 4096] int32 (int8-valued), scale/bias: [16384] f32

Sharding: W/scale/bias row-sharded 2048/core; x replicated (fed k-major as part of
layout prep); no collectives.

Per-core dataflow:
  - x.T (k-major f32) -> gpsimd cast-DMA f32->bf16 straight into resident SBUF
    tiles xT[kp, kt, m]  (contraction dim on partitions).
  - per n-tile (128 rows of W): gpsimd cast-DMA int32->bf16 (SDMA casts in the
    datapath), xbar DMA-transpose (Sync engine, transposes only -> no xbar/copy
    mode transitions) to wT[kp, kt, n].
  - 2 x 32 accumulating matmuls per n-tile (k-inner, one PSUM bank per 512-token
    chunk), PSUM f32 evicted via one DVE tensor_scalar (x*scale + bias, both
    per-partition scalars), output written as out.T [2048, 1024] f32 on Scalar
    HWDGE (keeps Sync xbar-only).
  - host: concat core outputs along n, transpose to [1024, 16384].
"""

import numpy as np

import concourse.tile as tile
from concourse import bacc, mybir
from concourse.bass_utils import run_bass_kernel_spmd

B, S, K, N = 2, 512, 4096, 16384
M = B * S              # 1024 tokens
NCORES = 8
NSH = N // NCORES      # 2048 out-features per core
P = 128
KT = K // P            # 32 k-tiles
NT = NSH // P          # 16 n-tiles per core
MCW = 512              # moving free dim per matmul (= one PSUM bank of f32)
MCH = M // MCW         # 2 token chunks
XG = 8                 # x load groups (4 k-tiles per DMA)


def build(w_bufs: int = 4, psum_bufs: int = 3):
    nc = bacc.Bacc("TRN2", target_bir_lowering=False, debug=False)
    xT_d = nc.dram_tensor("xT", [K, M], mybir.dt.float32, kind="ExternalInput")
    w_d = nc.dram_tensor("wq", [NSH, K], mybir.dt.int32, kind="ExternalInput")
    s_d = nc.dram_tensor("scale", [NSH, 1], mybir.dt.float32, kind="ExternalInput")
    b_d = nc.dram_tensor("bias", [NSH, 1], mybir.dt.float32, kind="ExternalInput")
    o_d = nc.dram_tensor("outT", [NSH, M], mybir.dt.float32, kind="ExternalOutput")

    kt_per_g = KT // XG
    with tile.TileContext(nc) as tc:
        with (
            tc.tile_pool(name="xT_pool", bufs=1) as xT_pool,
            tc.tile_pool(name="xstage", bufs=2) as xstage_pool,
            tc.tile_pool(name="wstage", bufs=w_bufs) as wstage_pool,
            tc.tile_pool(name="wT_pool", bufs=w_bufs) as wT_pool,
            tc.tile_pool(name="small", bufs=4) as small_pool,
            tc.tile_pool(name="osb", bufs=4) as osb_pool,
            tc.tile_pool(name="psum", bufs=psum_bufs, space="PSUM") as psum_pool,
        ):
            # x: f32 load on Scalar HWDGE (keeps the one SWDGE ring free for W
            # casts), DVE cast f32->bf16 into the resident k-major layout.
            xT = xT_pool.tile([P, KT, M], mybir.dt.bfloat16)
            for g in range(XG):
                xstg = xstage_pool.tile(
                    [P, kt_per_g, M], mybir.dt.float32, tag="xstg"
                )
                nc.scalar.dma_start(
                    out=xstg[:],
                    in_=xT_d.ap()[g * kt_per_g * P:(g + 1) * kt_per_g * P, :].rearrange(
                        "(kt p) m -> p kt m", p=P
                    ),
                )
                nc.vector.tensor_copy(
                    out=xT[:, g * kt_per_g:(g + 1) * kt_per_g, :], in_=xstg[:]
                )

            for nt in range(NT):
                w_sb = wstage_pool.tile([P, K], mybir.dt.bfloat16, tag="w_sb")
                nc.gpsimd.dma_start(out=w_sb[:], in_=w_d.ap()[nt * P:(nt + 1) * P, :])
                wT = wT_pool.tile([P, KT, P], mybir.dt.bfloat16, tag="wT")
                nc.sync.dma_start(out=wT[:], in_=w_sb[:], transpose=True)

                s_sb = small_pool.tile([P, 1], mybir.dt.float32, tag="s_sb")
                nc.scalar.dma_start(out=s_sb[:], in_=s_d.ap()[nt * P:(nt + 1) * P, :])
                b_sb = small_pool.tile([P, 1], mybir.dt.float32, tag="b_sb")
                nc.scalar.dma_start(out=b_sb[:], in_=b_d.ap()[nt * P:(nt + 1) * P, :])

                for c in range(MCH):
                    ps = psum_pool.tile(
                        [P, MCW], mybir.dt.float32, name=f"ps{c}", tag=f"ps{c}"
                    )
                    # k-inner: 32 back-to-back accumulating matmuls on one bank,
                    # 2D contiguous moving operand.
                    for kt in range(KT):
                        nc.tensor.matmul(
                            ps[:],
                            wT[:, kt, :],
                            xT[:, kt, c * MCW:(c + 1) * MCW],
                            start=(kt == 0),
                            stop=(kt == KT - 1),
                        )
                    o_sb = osb_pool.tile([P, MCW], mybir.dt.float32, tag="o_sb")
                    nc.vector.tensor_scalar(
                        out=o_sb[:],
                        in0=ps[:],
                        scalar1=s_sb[:],
                        scalar2=b_sb[:],
                        op0=mybir.AluOpType.mult,
                        op1=mybir.AluOpType.add,
                    )
                    nc.scalar.dma_start(
                        out=o_d.ap()[nt * P:(nt + 1) * P, c * MCW:(c + 1) * MCW],
                        in_=o_sb[:],
                    )
    nc.compile()
    return nc


def make_in_maps(x, weight_quant, scale, bias):
    x2T = np.ascontiguousarray(
        np.asarray(x, dtype=np.float32).reshape(M, K).T
    )  # [K, M] k-major replica
    scale = np.asarray(scale, dtype=np.float32).reshape(N, 1)
    bias = np.asarray(bias, dtype=np.float32).reshape(N, 1)
    wq = np.asarray(weight_quant, dtype=np.int32)
    in_maps = []
    for i in range(NCORES):
        sl = slice(i * NSH, (i + 1) * NSH)
        in_maps.append({
            "xT": x2T,
            "wq": np.ascontiguousarray(wq[sl]),
            "scale": np.ascontiguousarray(scale[sl]),
            "bias": np.ascontiguousarray(bias[sl]),
        })
    return in_maps


def gather_output(results):
    outT = np.concatenate([np.asarray(r["outT"]) for r in results], axis=0)  # [N, M]
    return np.ascontiguousarray(outT.T).reshape(B, S, N).astype(np.float32, copy=False)


def kernel(x, weight_quant, scale, bias):
    nc = build()
    in_maps = make_in_maps(x, weight_quant, scale, bias)
    res = run_bass_kernel_spmd(nc, in_maps, core_ids=list(range(NCORES)))
    return gather_output(res.results)


if __name__ == "__main__":
    rng = np.random.default_rng(0)
    x = rng.standard_normal((B, S, K), dtype=np.float32)
    wq = rng.integers(-128, 128, size=(N, K), dtype=np.int64).astype(np.int32)
    scale = rng.uniform(0.001, 0.02, size=(N,)).astype(np.float32)
    bias = rng.standard_normal((N,), dtype=np.float32)
    out = kernel(x=x, weight_quant=wq, scale=scale, bias=bias)
    w = wq.astype(np.float32) * scale[:, None]
    exp = x.reshape(M, K) @ w.T + bias
    err = np.abs(out.reshape(M, N) - exp).max() / np.abs(exp).max()
    print("self-check rel err:", err)
